# revision 16
# baseline (speedup 1.0000x reference)
"""Trainium2 Bass kernel for nn_AntisymMP (antisymmetric GNN message passing).

Strategy (8 NeuronCores, edge/graph parallelism):
 - Pairs (even/odd edge couples) are sorted by dst of the even edge and
   sharded so core c owns pairs whose even-dst lies in node band
   [2560c, 2560(c+1)).  Node features h are replicated (bf16), MLP weights
   replicated.
 - Antisymmetry algebra: with u=ha+hb, v=ha-hb, sE=e0+e1:
     zab/zba = t +- d + b_e1,  t = u@(W1a+W1b)/2 + sE@(W1c/2),  d = v@(W1a-W1b)/2
     mf = (silu(zab)-silu(zba)) @ W_e2          (b_e2 cancels)
   This halves the edge-MLP GEMM flops vs the reference.
 - Even messages (+mf -> even dst) are produced in dst-sorted order and
   reduced on the fly with signed one-hot matmuls into per-window PSUM
   accumulators (128-node windows), accumulated into an SBUF band
   accumulator (f32).
 - Odd messages (-mf -> random dst) are scattered (bf16) into a padded-CSR
   DRAM scratch via indirect DMA, then reduced window-by-window with one-hot
   matmuls into a full-N partial (bf16), ReduceScattered across the 8 cores.
 - Each core then runs the node MLP on its 2560-node band:
   out_h = h + nfn(cat[h, agg]),  agg = RS(odd partial) + local even band acc.
"""
import sys
import os

sys.path.insert(0, "/opt/trn_rl_repo")

import numpy as np
import ml_dtypes
from contextlib import ExitStack

from concourse import bass, bacc, tile, mybir
from concourse.bass_utils import run_bass_kernel_spmd
from concourse.masks import make_identity

BF = ml_dtypes.bfloat16
F32 = mybir.dt.float32
BF16 = mybir.dt.bfloat16
I16 = mybir.dt.int16
I32 = mybir.dt.int32

NC = 8
N, H, E = 20000, 256, 320000
NPAIR = E // 2
NPAD = 20480
BAND = NPAD // NC          # 2560
WINB = BAND // 128         # 20 local windows per band
NWIN = NPAD // 128         # 160 global windows

_cache = {}
SIM_SILU = False


def _silu_emit(nc, pool, out, in_, bias_col, tag):
    """out = silu(in_ + bias). Hardware path uses the ACT Silu LUT; the
    simulator lacks Silu so SIM_SILU mode decomposes via Sigmoid."""
    if not SIM_SILU:
        nc.scalar.activation(out, in_, mybir.ActivationFunctionType.Silu,
                             bias=bias_col)
    else:
        shp = list(in_.shape)
        sg = pool.tile(shp, F32, tag=tag + "_sg")
        nc.scalar.activation(sg[:], in_, mybir.ActivationFunctionType.Sigmoid,
                             bias=bias_col)
        xb = pool.tile(shp, F32, tag=tag + "_xb")
        nc.vector.tensor_scalar(out=xb[:], in0=in_, scalar1=bias_col,
                                scalar2=1.0, op0=mybir.AluOpType.add,
                                op1=mybir.AluOpType.mult)
        nc.vector.tensor_mul(out, xb[:], sg[:])


# ----------------------------------------------------------------------------
# host-side preparation: sorting, sharding, index/metadata arrays
# ----------------------------------------------------------------------------

def _wrap16(x):
    """[n] int array -> [128, n/16] int16 wrapped layout replicated over the
    8 gpsimd core blocks (16 partitions each)."""
    n = x.shape[0]
    assert n % 16 == 0
    a = x.reshape(n // 16, 16).T.astype(np.int16)   # [16, n/16]
    return np.tile(a, (8, 1))                        # [128, n/16]


def _colpack(x, rows=128):
    """[n] -> [128, n/128] column-per-tile layout (partition p, col t = x[128t+p])."""
    n = x.shape[0]
    assert n % rows == 0
    return np.ascontiguousarray(x.reshape(n // rows, rows).T)


def host_prep(h, e, ei):
    se = ei[0, 0::2].astype(np.int64)
    de = ei[1, 0::2].astype(np.int64)
    do = ei[1, 1::2].astype(np.int64)

    order = np.argsort(de, kind="stable")
    band = de[order] // BAND
    wloc = (de[order] % BAND) // 128

    percore = [order[band == c] for c in range(NC)]

    # even capacities per local window (max over cores)
    nE = np.zeros((NC, WINB), np.int64)
    for c in range(NC):
        w = wloc[band == c]
        np.add.at(nE[c], w, 1)
    capE = np.maximum(1, np.ceil(nE / 128).max(axis=0).astype(np.int64))
    cumE = np.concatenate([[0], np.cumsum(capE)])
    T = int(cumE[-1])
    Ppad = 128 * T

    # odd capacities per global window
    nO = np.zeros((NC, NWIN), np.int64)
    for c in range(NC):
        w = do[percore[c]] // 128
        np.add.at(nO[c], w, 1)
    capO = np.maximum(1, np.ceil(nO / 128).max(axis=0).astype(np.int64))
    cumO = np.concatenate([[0], np.cumsum(capO)])
    TO = int(cumO[-1])
    SCR = 128 * TO + 128     # + dump tile

    # slot -> local window map (static across cores)
    wl_of_tile = np.zeros(T, np.int64)
    for wl in range(WINB):
        wl_of_tile[cumE[wl]:cumE[wl + 1]] = wl

    cfg = dict(T=T, TO=TO, Ppad=Ppad, SCR=SCR,
               capE=tuple(int(x) for x in capE),
               capO=tuple(int(x) for x in capO))

    h_bf = np.ascontiguousarray(h.astype(BF))
    in_maps = []
    asm = []   # per-core pidx for output assembly
    for c in range(NC):
        pidx_pad = np.full(Ppad, -1, np.int64)
        pc = percore[c]
        wc = wloc[band == c]
        for wl in range(WINB):
            seg = pc[wc == wl]
            off = 128 * cumE[wl]
            pidx_pad[off:off + len(seg)] = seg
        real = pidx_pad >= 0
        pid = np.where(real, pidx_pad, 0)

        # e shard rows (2 per pair slot), pads zero
        rows = np.empty(2 * Ppad, np.int64)
        rows[0::2] = 2 * pid
        rows[1::2] = 2 * pid + 1
        e_in = e[rows].astype(np.float32)
        e_in[np.repeat(~real, 2)] = 0.0

        ga = np.where(real, se[pid], 0)
        gb = np.where(real, de[pid], 0)

        relE = np.where(real, de[pid] - (BAND * c + 128 * wl_of_tile[np.arange(Ppad) // 128]), -1)

        # odd scatter positions
        posO = np.full(Ppad, 0, np.int64)
        cnt = np.zeros(NWIN, np.int64)
        dw = do[pid] // 128
        for j in range(Ppad):
            if real[j]:
                w = dw[j]
                posO[j] = 128 * cumO[w] + cnt[w]
                cnt[w] += 1
            else:
                posO[j] = 128 * TO + (j % 128)

        # rel per scratch row
        relO_rows = np.full(128 * TO, -1.0, np.float32)
        rr = posO[real]
        relO_rows[rr] = (do[pid[real]] - 128 * (do[pid[real]] // 128)).astype(np.float32)

        nidx = BAND * c + np.arange(BAND)
        nidx = np.where(nidx < N, nidx, 0)

        # host-built one-hot tiles (bf16): even [128, T*128] (+1), odd [128, TO*128] (-1)
        io = np.arange(128, dtype=np.int64)
        relE_m = relE.reshape(T, 128)          # [T,128] per-tile rel
        ohE = np.zeros((T, 128, 128), np.float32)
        tt, pp = np.nonzero((relE_m >= 0) & (relE_m < 128))
        ohE[tt, pp, relE_m[tt, pp]] = 1.0
        relO_m = relO_rows.reshape(TO, 128).astype(np.int64)
        ohO = np.zeros((TO, 128, 128), np.float32)
        tt, pp = np.nonzero((relO_m >= 0) & (relO_m < 128))
        ohO[tt, pp, relO_m[tt, pp]] = -1.0
        ohE_u = np.ascontiguousarray(ohE.transpose(1, 0, 2).reshape(128, T * 128)).astype(BF)
        ohO_u = np.ascontiguousarray(ohO.transpose(1, 0, 2).reshape(128, TO * 128)).astype(BF)

        hband = np.zeros((BAND, H), np.float32)
        lo, hi = BAND * c, min(BAND * (c + 1), N)
        hband[: hi - lo] = h[lo:hi]

        in_maps.append(dict(
            e_in=e_in,
            h_bf=h_bf,
            hband=hband,
            gidx_a=_wrap16(ga),
            gidx_b=_wrap16(gb),
            nidx=_wrap16(nidx),
            ohE=ohE_u,
            ohO=ohO_u,
            posO=_colpack(posO.astype(np.int32)),
        ))
        asm.append((pidx_pad, real))
    return cfg, in_maps, asm


# ----------------------------------------------------------------------------
# device kernel builder
# ----------------------------------------------------------------------------

def build(cfg):
    import os as _os
    DBG = set(_os.environ.get("KDBG", "").split(","))

    T, TO, Ppad, SCR = cfg["T"], cfg["TO"], cfg["Ppad"], cfg["SCR"]
    capE, capO = cfg["capE"], cfg["capO"]
    cumE = np.concatenate([[0], np.cumsum(capE)]).astype(int)
    cumO = np.concatenate([[0], np.cumsum(capO)]).astype(int)

    nc = bacc.Bacc("TRN2", target_bir_lowering=False, debug=False, num_devices=NC)

    e_in = nc.declare_dram_parameter("e_in", [2 * Ppad, H], F32, isOutput=False)
    h_bf = nc.declare_dram_parameter("h_bf", [N, H], BF16, isOutput=False)
    hband = nc.declare_dram_parameter("hband", [BAND, H], F32, isOutput=False)
    gidx_a = nc.declare_dram_parameter("gidx_a", [128, Ppad // 16], I16, isOutput=False)
    gidx_b = nc.declare_dram_parameter("gidx_b", [128, Ppad // 16], I16, isOutput=False)
    nidx = nc.declare_dram_parameter("nidx", [128, BAND // 16], I16, isOutput=False)
    ohE = nc.declare_dram_parameter("ohE", [128, T * 128], BF16, isOutput=False)
    ohO = nc.declare_dram_parameter("ohO", [128, TO * 128], BF16, isOutput=False)
    posO = nc.declare_dram_parameter("posO", [128, T], I32, isOutput=False)
    we1 = nc.declare_dram_parameter("we1", [3 * H, H], F32, isOutput=False)
    we2 = nc.declare_dram_parameter("we2", [H, H], F32, isOutput=False)
    wn1 = nc.declare_dram_parameter("wn1", [2 * H, H], F32, isOutput=False)
    wn2 = nc.declare_dram_parameter("wn2", [H, H], F32, isOutput=False)
    be1c = nc.declare_dram_parameter("be1c", [128, 2], F32, isOutput=False)
    bn1c = nc.declare_dram_parameter("bn1c", [128, 2], F32, isOutput=False)
    bn2b = nc.declare_dram_parameter("bn2b", [128, H], F32, isOutput=False)

    e2_out = nc.declare_dram_parameter("e2_out", [2 * Ppad, H], F32, isOutput=True)
    oh_out = nc.declare_dram_parameter("oh_out", [BAND, H], F32, isOutput=True)
    scr = nc.declare_dram_parameter("scr", [SCR, H], BF16, isOutput=True)

    partial = nc.dram_tensor("partial", [NPAD, H], BF16)
    rs_out = nc.dram_tensor("rs_out", [BAND, H], BF16)

    GB = 2 if "gb2" in DBG else 4   # subtiles per gather batch

    with ExitStack() as ctx:
        tc = ctx.enter_context(tile.TileContext(nc))

        const = ctx.enter_context(tc.tile_pool(name="const", bufs=1))
        wpool = ctx.enter_context(tc.tile_pool(name="wpool", bufs=1))

        # ---- constants / aux ----
        identb = const.tile([128, 128], BF16)
        make_identity(nc, identb[:])

        posO_sb = const.tile([128, T], I32)
        nc.sync.dma_start(posO_sb[:], posO[:])
        gia_sb = const.tile([128, Ppad // 16], I16)
        nc.sync.dma_start(gia_sb[:], gidx_a[:])
        gib_sb = const.tile([128, Ppad // 16], I16)
        nc.sync.dma_start(gib_sb[:], gidx_b[:])
        be1_sb = const.tile([128, 2], F32)
        nc.sync.dma_start(be1_sb[:], be1c[:])
        bn1_sb = const.tile([128, 2], F32)
        nc.sync.dma_start(bn1_sb[:], bn1c[:])
        bn2_sb = const.tile([128, H], F32)
        nc.sync.dma_start(bn2_sb[:], bn2b[:])

        band_acc = const.tile([128, WINB, H], F32)

        # ---- weight prep (bf16) ----
        wtmp = wpool.tile([128, 6, H], F32)
        for k in range(6):
            nc.sync.dma_start(wtmp[:, k, :], we1[128 * k:128 * (k + 1), :])
        w1s = wpool.tile([128, 4, H], BF16)   # kc 0,1 = (W1a+W1b)/2 ; kc 2,3 = W1c/2
        wdf = wpool.tile([128, 2, H], BF16)
        wsc = wpool.tile([128, 2, H], F32)
        for kc in range(2):
            nc.vector.tensor_add(wsc[:, kc, :], wtmp[:, kc, :], wtmp[:, 2 + kc, :])
            nc.scalar.activation(w1s[:, kc, :], wsc[:, kc, :],
                                 mybir.ActivationFunctionType.Copy, scale=0.5)
            nc.scalar.activation(w1s[:, 2 + kc, :], wtmp[:, 4 + kc, :],
                                 mybir.ActivationFunctionType.Copy, scale=0.5)
        wsc2 = wpool.tile([128, 2, H], F32)
        for kc in range(2):
            nc.vector.tensor_sub(wsc2[:, kc, :], wtmp[:, kc, :], wtmp[:, 2 + kc, :])
            nc.scalar.activation(wdf[:, kc, :], wsc2[:, kc, :],
                                 mybir.ActivationFunctionType.Copy, scale=0.5)
        w2s = wpool.tile([128, 2, H], BF16)
        wn1s = wpool.tile([128, 4, H], BF16)
        wn2s = wpool.tile([128, 2, H], BF16)
        wtmp2 = wpool.tile([128, 4, H], F32)
        for kc in range(2):
            nc.sync.dma_start(wtmp2[:, kc, :], we2[128 * kc:128 * (kc + 1), :])
            nc.scalar.activation(w2s[:, kc, :], wtmp2[:, kc, :],
                                 mybir.ActivationFunctionType.Copy)
        wtmp3 = wpool.tile([128, 4, H], F32)
        for kc in range(4):
            nc.sync.dma_start(wtmp3[:, kc, :], wn1[128 * kc:128 * (kc + 1), :])
            nc.scalar.activation(wn1s[:, kc, :], wtmp3[:, kc, :],
                                 mybir.ActivationFunctionType.Copy)
        wtmp4 = wpool.tile([128, 2, H], F32)
        for kc in range(2):
            nc.sync.dma_start(wtmp4[:, kc, :], wn2[128 * kc:128 * (kc + 1), :])
            nc.scalar.activation(wn2s[:, kc, :], wtmp4[:, kc, :],
                                 mybir.ActivationFunctionType.Copy)

        # ================= phase A: pair pipeline =================
        with (
            tc.tile_pool(name="ght", bufs=2) as ghtp,
            tc.tile_pool(name="ework", bufs=3) as ep,
            tc.tile_pool(name="work", bufs=3) as wk,
            tc.tile_pool(name="msg", bufs=3) as msgp,
            tc.tile_pool(name="ohp", bufs=2) as ohp,
            tc.tile_pool(name="ps_t", bufs=2, space="PSUM") as ps_t,
            tc.tile_pool(name="ps_d", bufs=1, space="PSUM") as ps_d,
            tc.tile_pool(name="ps_mf", bufs=2, space="PSUM") as ps_mf,
            tc.tile_pool(name="ps_tr", bufs=1, space="PSUM") as ps_tr,
            tc.tile_pool(name="ps_w", bufs=2, space="PSUM") as ps_w,
        ):
            ght_a = ght_b = None
            psw = None
            n_mt = (T + 1) // 2
            for mt in range(n_mt):
                t0 = 2 * mt
                nsub = min(2, T - t0)         # subtiles in this macrotile
                PW = 128 * nsub               # pairs in macrotile

                if t0 % GB == 0:
                    nb = min(GB, T - t0)      # subtiles in gather batch
                    ght_a = ghtp.tile([128, 2, 128 * nb], BF16, tag="ga")
                    ght_b = ghtp.tile([128, 2, 128 * nb], BF16, tag="gb")
                    for gt, dst in ((gia_sb, ght_a), (gib_sb, ght_b)):
                        nc.gpsimd.dma_gather(
                            out_ap=dst[:],
                            in_ap=h_bf[:],
                            idxs_ap=gt[:, 8 * t0: 8 * (t0 + nb)],
                            num_idxs=128 * nb,
                            num_idxs_reg=128 * nb,
                            elem_size=H,
                            transpose=True,
                        )
                if t0 % GB == 0:
                    nb2 = min(GB, T - t0)
                    ohE_sb = ohp.tile([128, 128 * nb2], BF16, tag="ohe")
                    nc.sync.dma_start(ohE_sb[:], ohE[:, 128 * t0:128 * (t0 + nb2)])
                goff = 128 * (t0 % GB)

                # e rows for macrotile: [128, nsub, 2, H]
                e_sb = ep.tile([128, 2, 2, H], F32, tag="e")
                src = e_in[512 * mt: 512 * mt + 256 * nsub, :]
                nc.sync.dma_start(
                    e_sb[:, 0:nsub, :, :],
                    src.rearrange("(s p c) h -> p s c h", s=nsub, p=128, c=2),
                )

                # sE = e0+e1 (bf16; the 0.5 is folded into W1c)
                se_b = wk.tile([128, 2, H], BF16, tag="se")
                nc.vector.tensor_add(se_b[:, 0:nsub, :], e_sb[:, 0:nsub, 0, :],
                                     e_sb[:, 0:nsub, 1, :])

                # seT via PE transpose
                seT = wk.tile([128, 2, PW], BF16, tag="seT")
                for s in range(nsub):
                    for cch in range(2):
                        pst = ps_tr.tile([128, 128], BF16, space="PSUM", tag="tr")
                        nc.tensor.transpose(pst[:], se_b[:, s, 128 * cch:128 * (cch + 1)], identb[:])
                        nc.scalar.activation(seT[:, cch, 128 * s:128 * (s + 1)], pst[:],
                                             mybir.ActivationFunctionType.Copy)

                # u/v
                ga = ght_a[:, :, goff:goff + PW]
                gb = ght_b[:, :, goff:goff + PW]
                uT = wk.tile([128, 2, PW], BF16, tag="uT")
                vT = wk.tile([128, 2, PW], BF16, tag="vT")
                nc.vector.tensor_add(uT[:], ga, gb)
                nc.vector.tensor_sub(vT[:], ga, gb)

                # GEMM1a: t = u @ Wsum' + sE @ (W1c/2)   -> [Hout(2x128), PW]
                pt = ps_t.tile([128, 2, PW], F32, space="PSUM", tag="pt")
                for mc in range(2):
                    for kc in range(4):
                        rhs = uT[:, kc, :] if kc < 2 else seT[:, kc - 2, :]
                        nc.tensor.matmul(pt[:, mc, :], lhsT=w1s[:, kc, 128 * mc:128 * (mc + 1)],
                                         rhs=rhs, start=(kc == 0), stop=(kc == 3))
                # GEMM1b: d = v @ Wdiff'
                pd = ps_d.tile([128, 2, PW], F32, space="PSUM", tag="pd")
                for mc in range(2):
                    for kc in range(2):
                        nc.tensor.matmul(pd[:, mc, :], lhsT=wdf[:, kc, 128 * mc:128 * (mc + 1)],
                                         rhs=vT[:, kc, :], start=(kc == 0), stop=(kc == 1))

                pdc = wk.tile([128, 2, PW], F32, tag="pdc")
                for mc in range(2):
                    nc.scalar.activation(pdc[:, mc, :], pd[:, mc, 0:PW],
                                         mybir.ActivationFunctionType.Copy)
                zab = wk.tile([128, 2, PW], F32, tag="zab")
                zba = wk.tile([128, 2, PW], F32, tag="zba")
                nc.vector.tensor_add(zab[:], pt[:, :, 0:PW], pdc[:])
                nc.vector.tensor_sub(zba[:], pt[:, :, 0:PW], pdc[:])
                gab = wk.tile([128, 2, PW], F32, tag="gab")
                gba = wk.tile([128, 2, PW], F32, tag="gba")
                for mc in range(2):
                    _silu_emit(nc, wk, gab[:, mc, :], zab[:, mc, :],
                               be1_sb[:, mc:mc + 1], "sa")
                    _silu_emit(nc, wk, gba[:, mc, :], zba[:, mc, :],
                               be1_sb[:, mc:mc + 1], "sb")
                gdf = wk.tile([128, 2, PW], BF16, tag="gdf")
                nc.vector.tensor_sub(gdf[:], gab[:], gba[:])

                e2_sb = ep.tile([128, 2, 2, H], F32, tag="e2")
                pmf = ps_mf.tile([128, 2, H], F32, space="PSUM", tag="mf")
                mf_b = msgp.tile([128, 2, H], BF16, tag="mfb")
                for s in range(nsub):
                    # GEMM2: mf = g @ W_e2  -> [128 pairs, H]
                    for kc in range(2):
                        nc.tensor.matmul(pmf[:, s, :], lhsT=gdf[:, kc, 128 * s:128 * (s + 1)],
                                         rhs=w2s[:, kc, :], start=(kc == 0), stop=(kc == 1))
                nc.vector.tensor_copy(mf_b[:, 0:nsub, :], pmf[:, 0:nsub, :])
                nc.vector.tensor_add(e2_sb[:, 0:nsub, 0, :], e_sb[:, 0:nsub, 0, :],
                                     pmf[:, 0:nsub, :])
                nc.vector.tensor_sub(e2_sb[:, 0:nsub, 1, :], e_sb[:, 0:nsub, 1, :],
                                     pmf[:, 0:nsub, :])
                for s in range(nsub):
                    t = t0 + s
                    # even scatter (fused, +1 one-hot from host)
                    wl = int(np.searchsorted(cumE, t, side="right") - 1)
                    if "noeven" in DBG:
                        wl = None
                    first = (wl is not None) and (t == cumE[wl])
                    last = (wl is not None) and (t == cumE[wl + 1] - 1)
                    if first:
                        psw = ps_w.tile([128, H], F32, space="PSUM", tag="win")
                    if wl is not None:
                        nc.tensor.matmul(psw[:], lhsT=ohE_sb[:, 128 * (t % GB):128 * (t % GB + 1)],
                                         rhs=mf_b[:, s, :],
                                         start=first, stop=last, skip_group_check=True)
                        if last:
                            nc.vector.tensor_copy(band_acc[:, wl, :], psw[:])

                # odd scatter to scratch (one call per subtile: multi-column
                # offset batching silently drops rows on HW)
                if "noscat" not in DBG:
                    for s_ in range(nsub):
                        nc.gpsimd.indirect_dma_start(
                            out=scr[:],
                            out_offset=bass.IndirectOffsetOnAxis(
                                ap=posO_sb[:, t0 + s_:t0 + s_ + 1], axis=0),
                            in_=mf_b[:, s_, :],
                            in_offset=None,
                        )

                dstv = e2_out[512 * mt: 512 * mt + 256 * nsub, :]
                nc.sync.dma_start(
                    dstv.rearrange("(s p c) h -> p s c h", s=nsub, p=128, c=2),
                    e2_sb[:, 0:nsub, :, :],
                )

        # ================= phase B: odd window reduction =================
        if "nophaseb" not in DBG:
         with (
            tc.tile_pool(name="scrd", bufs=3) as scrp,
            tc.tile_pool(name="pout", bufs=2) as poutp,
            tc.tile_pool(name="ps_b", bufs=2, space="PSUM") as ps_b,
         ):
            SB_ = 8   # scratch tiles per load batch
            WB = 8    # windows per output DMA batch
            w_of_st = np.zeros(TO, np.int64)
            for w in range(NWIN):
                w_of_st[cumO[w]:cumO[w + 1]] = w
            sc_sb = None
            oh_sb = None
            pb = None
            psb = None
            for st in range(TO):
                if st % SB_ == 0:
                    nb3 = min(SB_, TO - st)
                    sc_sb = scrp.tile([128, SB_, H], BF16, tag="sc")
                    nc.sync.dma_start(
                        sc_sb[:, 0:nb3, :],
                        scr[128 * st:128 * (st + nb3), :].rearrange(
                            "(s p) h -> p s h", s=nb3, p=128),
                    )
                    oh_sb = scrp.tile([128, SB_ * 128], BF16, tag="ohb")
                    nc.sync.dma_start(oh_sb[:, 0:128 * nb3],
                                      ohO[:, 128 * st:128 * (st + nb3)])
                w = int(w_of_st[st])
                if st == cumO[w]:
                    psb = ps_b.tile([128, H], F32, space="PSUM", tag="bw")
                nc.tensor.matmul(psb[:], lhsT=oh_sb[:, 128 * (st % SB_):128 * (st % SB_ + 1)],
                                 rhs=sc_sb[:, st % SB_, :],
                                 start=(st == cumO[w]), stop=(st == cumO[w + 1] - 1),
                                 skip_group_check=True)
                if st == cumO[w + 1] - 1:
                    if w % WB == 0:
                        pb = poutp.tile([128, WB, H], BF16, tag="pb")
                    nc.scalar.activation(pb[:, w % WB, :], psb[:],
                                         mybir.ActivationFunctionType.Copy)
                    if w % WB == WB - 1:
                        wg = w // WB
                        dst = partial[128 * WB * wg: 128 * WB * (wg + 1), :]
                        nc.sync.dma_start(
                            dst.rearrange("(s p) h -> p s h", s=WB, p=128),
                            pb[:],
                        )

        # ================= reduce-scatter =================
        if "nors" not in DBG:
         nc.gpsimd.collective_compute(
            "ReduceScatter",
            mybir.AluOpType.add,
            replica_groups=[list(range(NC))],
            ins=[partial[:]],
            outs=[rs_out[:]],
         )

        # ================= phase C: node MLP on band =================
        if "nophasec" not in DBG:
         with (
            tc.tile_pool(name="nfw", bufs=3) as nf,
            tc.tile_pool(name="nfT", bufs=1) as nfT,
            tc.tile_pool(name="ps_z", bufs=2, space="PSUM") as ps_z,
            tc.tile_pool(name="ps_o", bufs=2, space="PSUM") as ps_o,
            tc.tile_pool(name="ps_tr", bufs=2, space="PSUM") as ps_tr,
        ):
            nix = nf.tile([128, BAND // 16], I16, tag="nix")
            nc.sync.dma_start(nix[:], nidx[:])
            GC = 256 if BAND >= 256 else 128
            hTs = []
            for i in range(BAND // GC):
                hTc = nfT.tile([128, 2, GC], BF16, tag=f"hT{i}")
                nc.gpsimd.dma_gather(
                    out_ap=hTc[:], in_ap=h_bf[:],
                    idxs_ap=nix[:, (GC // 16) * i:(GC // 16) * (i + 1)],
                    num_idxs=GC, num_idxs_reg=GC, elem_size=H, transpose=True,
                )
                hTs.append(hTc)
            aggT = nfT.tile([128, 2, BAND], BF16, tag="aggT")
            agg_b = nfT.tile([128, WINB, H], BF16, tag="aggb")
            rs_sb = nfT.tile([128, WINB, H], BF16, tag="rs")
            nc.sync.dma_start(
                rs_sb[:], rs_out[:].rearrange("(s p) h -> p s h", s=WINB, p=128))
            rs_f = nfT.tile([128, WINB, H], F32, tag="rsf")
            nc.scalar.activation(rs_f[:], rs_sb[:], mybir.ActivationFunctionType.Copy)
            nc.vector.tensor_add(agg_b[:], band_acc[:], rs_f[:])
            for i in range(WINB):
                for cch in range(2):
                    ptr = ps_tr.tile([128, 128], BF16, space="PSUM", tag="ntr")
                    nc.tensor.transpose(ptr[:], agg_b[:, i, 128 * cch:128 * (cch + 1)], identb[:])
                    nc.scalar.activation(aggT[:, cch, 128 * i:128 * (i + 1)], ptr[:],
                                         mybir.ActivationFunctionType.Copy)

            NF = min(256, BAND)
            for nfc in range(BAND // NF):
                pz = ps_z.tile([128, 2, NF], F32, space="PSUM", tag="pz")
                for mc in range(2):
                    for kc in range(4):
                        rhs = (hTs[(NF * nfc) // GC][:, kc - 2 if False else kc, (NF * nfc) % GC:(NF * nfc) % GC + NF] if kc < 2
                               else aggT[:, kc - 2, NF * nfc:NF * (nfc + 1)])
                        nc.tensor.matmul(pz[:, mc, :], lhsT=wn1s[:, kc, 128 * mc:128 * (mc + 1)],
                                         rhs=rhs, start=(kc == 0), stop=(kc == 3))
                gn = nf.tile([128, 2, NF], BF16, tag="gn")
                for mc in range(2):
                    _silu_emit(nc, nf, gn[:, mc, :], pz[:, mc, :],
                               bn1_sb[:, mc:mc + 1], "sn")
                for i4 in range(NF // 128):
                    row0 = NF * nfc + 128 * i4
                    po = ps_o.tile([128, H], F32, space="PSUM", tag="po")
                    for kc in range(2):
                        nc.tensor.matmul(po[:], lhsT=gn[:, kc, 128 * i4:128 * (i4 + 1)],
                                         rhs=wn2s[:, kc, :], start=(kc == 0), stop=(kc == 1))
                    hb_sb = nf.tile([128, H], F32, tag="hb")
                    nc.sync.dma_start(hb_sb[:], hband[row0:row0 + 128, :])
                    o1 = nf.tile([128, H], F32, tag="o1")
                    nc.vector.tensor_add(o1[:], po[:], hb_sb[:])
                    o2 = nf.tile([128, H], F32, tag="o2")
                    nc.vector.tensor_add(o2[:], o1[:], bn2_sb[:])
                    nc.sync.dma_start(oh_out[row0:row0 + 128, :], o2[:])
                    del hb_sb, o1, o2

    nc.compile()
    return nc


# ----------------------------------------------------------------------------
# entry point
# ----------------------------------------------------------------------------

def kernel(h, e, W_e1, b_e1, W_e2, b_e2, W_n1, b_n1, W_n2, b_n2, ei, _trace=False):
    h = np.asarray(h, dtype=np.float32)
    e = np.asarray(e, dtype=np.float32)
    ei_np = np.asarray(ei)
    W_e1 = np.asarray(W_e1, dtype=np.float32)
    b_e1 = np.asarray(b_e1, dtype=np.float32)
    W_e2 = np.asarray(W_e2, dtype=np.float32)
    W_n1 = np.asarray(W_n1, dtype=np.float32)
    b_n1 = np.asarray(b_n1, dtype=np.float32)
    W_n2 = np.asarray(W_n2, dtype=np.float32)
    b_n2 = np.asarray(b_n2, dtype=np.float32)
    b_e2 = np.asarray(b_e2, dtype=np.float32)

    h2, e2d = h[0], e[0]
    cfg, in_maps, asm = host_prep(h2, e2d, ei_np.astype(np.int64))

    be1c = np.ascontiguousarray(b_e1.reshape(2, 128).T)
    bn1c = np.ascontiguousarray(b_n1.reshape(2, 128).T)
    bn2b = np.tile(b_n2.reshape(1, H), (128, 1)).astype(np.float32)

    for m in in_maps:
        m.update(dict(we1=W_e1, we2=W_e2, wn1=W_n1, wn2=W_n2,
                      be1c=be1c, bn1c=bn1c, bn2b=bn2b))

    key = (cfg["T"], cfg["TO"], cfg["SCR"], cfg["capE"], cfg["capO"], os.environ.get("KDBG",""))
    if key not in _cache:
        _cache[key] = build(cfg)
    nc = _cache[key]

    res = run_bass_kernel_spmd(nc, in_maps, core_ids=list(range(NC)), trace=_trace)
    kernel._last_results = res

    out_h = np.empty((1, N, H), np.float32)
    e2 = np.empty((1, E, H), np.float32)
    for c in range(NC):
        r = res.results[c]
        lo, hi = BAND * c, min(BAND * (c + 1), N)
        out_h[0, lo:hi] = r["oh_out"][: hi - lo]
        pidx_pad, real = asm[c]
        pid = pidx_pad[real]
        ero = r["e2_out"].reshape(-1, 2, H)[real]
        e2[0, 2 * pid] = ero[:, 0]
        e2[0, 2 * pid + 1] = ero[:, 1]
    # add b_e2 to both sides?  No: b_e2 cancels in mf; e2 = e + msgs exactly.
    return out_h, e2


# revision 22
# speedup vs baseline: 1.5612x; 1.5612x over previous
"""Trainium2 Bass kernel for nn_AntisymMP (antisymmetric GNN message passing).

Strategy (8 NeuronCores, edge/graph parallelism):
 - Pairs (even/odd edge couples) are sorted by dst of the even edge and
   sharded so core c owns pairs whose even-dst lies in node band
   [2560c, 2560(c+1)).  Node features h are replicated (bf16), MLP weights
   replicated.
 - Antisymmetry algebra: with u=ha+hb, v=ha-hb, sE=e0+e1:
     zab/zba = t +- d + b_e1,  t = u@(W1a+W1b)/2 + sE@(W1c/2),  d = v@(W1a-W1b)/2
     mf = (silu(zab)-silu(zba)) @ W_e2          (b_e2 cancels)
   This halves the edge-MLP GEMM flops vs the reference.
 - Even messages (+mf -> even dst) are produced in dst-sorted order and
   reduced on the fly with signed one-hot matmuls into per-window PSUM
   accumulators (128-node windows), accumulated into an SBUF band
   accumulator (f32).
 - Odd messages (-mf -> random dst) are scattered (bf16) into a padded-CSR
   DRAM scratch via indirect DMA, then reduced window-by-window with one-hot
   matmuls into a full-N partial (bf16), ReduceScattered across the 8 cores.
 - Each core then runs the node MLP on its 2560-node band:
   out_h = h + nfn(cat[h, agg]),  agg = RS(odd partial) + local even band acc.
"""
import sys
import os

sys.path.insert(0, "/opt/trn_rl_repo")

import numpy as np
import ml_dtypes
from contextlib import ExitStack

from concourse import bass, bacc, tile, mybir
from concourse.bass_utils import run_bass_kernel_spmd
from concourse.masks import make_identity

BF = ml_dtypes.bfloat16
F32 = mybir.dt.float32
BF16 = mybir.dt.bfloat16
I16 = mybir.dt.int16
I32 = mybir.dt.int32

NC = 8
N, H, E = 20000, 256, 320000
NPAIR = E // 2
NPAD = 20480
BAND = NPAD // NC          # 2560
WINB = BAND // 128         # 20 local windows per band
NWIN = NPAD // 128         # 160 global windows

_cache = {}
SIM_SILU = False


def _silu_emit(nc, pool, out, in_, bias_col, tag):
    """out = silu(in_ + bias). Hardware path uses the ACT Silu LUT; the
    simulator lacks Silu so SIM_SILU mode decomposes via Sigmoid."""
    if not SIM_SILU:
        nc.scalar.activation(out, in_, mybir.ActivationFunctionType.Silu,
                             bias=bias_col)
    else:
        shp = list(in_.shape)
        sg = pool.tile(shp, F32, tag=tag + "_sg")
        nc.scalar.activation(sg[:], in_, mybir.ActivationFunctionType.Sigmoid,
                             bias=bias_col)
        xb = pool.tile(shp, F32, tag=tag + "_xb")
        nc.vector.tensor_scalar(out=xb[:], in0=in_, scalar1=bias_col,
                                scalar2=1.0, op0=mybir.AluOpType.add,
                                op1=mybir.AluOpType.mult)
        nc.vector.tensor_mul(out, xb[:], sg[:])


# ----------------------------------------------------------------------------
# host-side preparation: sorting, sharding, index/metadata arrays
# ----------------------------------------------------------------------------

def _wrap16(x):
    """[n] int array -> [128, n/16] int16 wrapped layout replicated over the
    8 gpsimd core blocks (16 partitions each)."""
    n = x.shape[0]
    assert n % 16 == 0
    a = x.reshape(n // 16, 16).T.astype(np.int16)   # [16, n/16]
    return np.tile(a, (8, 1))                        # [128, n/16]


def _colpack(x, rows=128):
    """[n] -> [128, n/128] column-per-tile layout (partition p, col t = x[128t+p])."""
    n = x.shape[0]
    assert n % rows == 0
    return np.ascontiguousarray(x.reshape(n // rows, rows).T)


def host_prep(h, e, ei):
    se = ei[0, 0::2].astype(np.int64)
    de = ei[1, 0::2].astype(np.int64)
    do = ei[1, 1::2].astype(np.int64)

    order = np.argsort(de, kind="stable")
    band = de[order] // BAND
    wloc = (de[order] % BAND) // 128

    percore = [order[band == c] for c in range(NC)]

    # even capacities per local window (max over cores)
    nE = np.zeros((NC, WINB), np.int64)
    for c in range(NC):
        w = wloc[band == c]
        np.add.at(nE[c], w, 1)
    capE = np.maximum(1, np.ceil(nE / 128).max(axis=0).astype(np.int64))
    cumE = np.concatenate([[0], np.cumsum(capE)])
    T = int(cumE[-1])
    Ppad = 128 * T

    # odd capacities per global window
    nO = np.zeros((NC, NWIN), np.int64)
    for c in range(NC):
        w = do[percore[c]] // 128
        np.add.at(nO[c], w, 1)
    capO = np.maximum(1, np.ceil(nO / 128).max(axis=0).astype(np.int64))
    cumO = np.concatenate([[0], np.cumsum(capO)])
    TO = int(cumO[-1])
    # split windows into two scratch halves so row ids fit int16
    HW = int(np.searchsorted(cumO, TO // 2))
    TA, TB = int(cumO[HW]), TO - int(cumO[HW])
    SCRA = 128 * TA + 128
    SCRB = 128 * TB + 128
    assert SCRA <= 32767 and SCRB <= 32767, (SCRA, SCRB)
    SCR = SCRA + SCRB  # kept for cfg compat

    # slot -> local window map (static across cores)
    wl_of_tile = np.zeros(T, np.int64)
    for wl in range(WINB):
        wl_of_tile[cumE[wl]:cumE[wl + 1]] = wl

    cfg = dict(T=T, TO=TO, Ppad=Ppad, SCR=SCR, HW=HW, SCRA=SCRA, SCRB=SCRB,
               capE=tuple(int(x) for x in capE),
               capO=tuple(int(x) for x in capO))

    h_bf = np.ascontiguousarray(h.astype(BF))
    in_maps = []
    asm = []   # per-core pidx for output assembly
    for c in range(NC):
        pidx_pad = np.full(Ppad, -1, np.int64)
        pc = percore[c]
        wc = wloc[band == c]
        for wl in range(WINB):
            seg = pc[wc == wl]
            off = 128 * cumE[wl]
            pidx_pad[off:off + len(seg)] = seg
        real = pidx_pad >= 0
        pid = np.where(real, pidx_pad, 0)

        # e shard rows (2 per pair slot), pads zero
        rows = np.empty(2 * Ppad, np.int64)
        rows[0::2] = 2 * pid
        rows[1::2] = 2 * pid + 1
        e_in = e[rows].astype(np.float32)
        e_in[np.repeat(~real, 2)] = 0.0

        ga = np.where(real, se[pid], 0)
        gb = np.where(real, de[pid], 0)

        relE = np.where(real, de[pid] - (BAND * c + 128 * wl_of_tile[np.arange(Ppad) // 128]), -1)

        # odd scatter positions (two halves, int16-safe)
        posA = np.empty(Ppad, np.int64)
        posB = np.empty(Ppad, np.int64)
        relO_rows = np.full(128 * TO, -1.0, np.float32)
        cnt = np.zeros(NWIN, np.int64)
        dw = do[pid] // 128
        for j in range(Ppad):
            w = dw[j]
            if real[j]:
                g = 128 * cumO[w] + cnt[w]
                relO_rows[g] = do[pid[j]] - 128 * w
                cnt[w] += 1
                if w < HW:
                    posA[j] = g
                    posB[j] = 128 * TB + (j % 128)
                else:
                    posA[j] = 128 * TA + (j % 128)
                    posB[j] = g - 128 * TA
            else:
                posA[j] = 128 * TA + (j % 128)
                posB[j] = 128 * TB + (j % 128)

        nidx = BAND * c + np.arange(BAND)
        nidx = np.where(nidx < N, nidx, 0)

        # host-built one-hot tiles (bf16): even [128, T*128] (+1), odd [128, TO*128] (-1)
        io = np.arange(128, dtype=np.int64)
        relE_m = relE.reshape(T, 128)          # [T,128] per-tile rel
        ohE = np.zeros((T, 128, 128), np.float32)
        tt, pp = np.nonzero((relE_m >= 0) & (relE_m < 128))
        ohE[tt, pp, relE_m[tt, pp]] = 1.0
        relO_m = relO_rows.reshape(TO, 128).astype(np.int64)
        ohO = np.zeros((TO, 128, 128), np.float32)
        tt, pp = np.nonzero((relO_m >= 0) & (relO_m < 128))
        ohO[tt, pp, relO_m[tt, pp]] = -1.0
        ohE_u = np.ascontiguousarray(ohE.transpose(1, 0, 2).reshape(128, T * 128)).astype(BF)
        ohO_u = np.ascontiguousarray(ohO.transpose(1, 0, 2).reshape(128, TO * 128)).astype(BF)
        ohT_u = np.ascontiguousarray(ohE.transpose(2, 0, 1).reshape(128, T * 128)).astype(BF)

        _lo, _hi = BAND * c, min(BAND * (c + 1), N)
        hbb = np.zeros((BAND, H), BF)
        hbb[: _hi - _lo] = h_bf[_lo:_hi]

        hband = np.zeros((BAND, H), np.float32)
        lo, hi = BAND * c, min(BAND * (c + 1), N)
        hband[: hi - lo] = h[lo:hi]

        in_maps.append(dict(
            e_in=e_in,
            h_bf=h_bf,
            hband=hband,
            gidx_a=_wrap16(ga),

            nidx=_wrap16(nidx),
            ohE=ohE_u,
            ohO=ohO_u,
            ohT=ohT_u,
            hbb=hbb,
            posA=_wrap16(posA),
            posB=_wrap16(posB),
        ))
        asm.append((pidx_pad, real))
    return cfg, in_maps, asm


# ----------------------------------------------------------------------------
# device kernel builder
# ----------------------------------------------------------------------------

def build(cfg):
    import os as _os
    DBG = set(_os.environ.get("KDBG", "").split(","))

    T, TO, Ppad, SCR = cfg["T"], cfg["TO"], cfg["Ppad"], cfg["SCR"]
    capE, capO = cfg["capE"], cfg["capO"]
    cumE = np.concatenate([[0], np.cumsum(capE)]).astype(int)
    cumO = np.concatenate([[0], np.cumsum(capO)]).astype(int)

    nc = bacc.Bacc("TRN2", target_bir_lowering=False, debug=False, num_devices=NC)

    e_in = nc.declare_dram_parameter("e_in", [2 * Ppad, H], F32, isOutput=False)
    h_bf = nc.declare_dram_parameter("h_bf", [N, H], BF16, isOutput=False)
    hband = nc.declare_dram_parameter("hband", [BAND, H], F32, isOutput=False)
    gidx_a = nc.declare_dram_parameter("gidx_a", [128, Ppad // 16], I16, isOutput=False)
    hbb = nc.declare_dram_parameter("hbb", [BAND, H], BF16, isOutput=False)
    nidx = nc.declare_dram_parameter("nidx", [128, BAND // 16], I16, isOutput=False)
    ohE = nc.declare_dram_parameter("ohE", [128, T * 128], BF16, isOutput=False)
    ohO = nc.declare_dram_parameter("ohO", [128, TO * 128], BF16, isOutput=False)
    ohT = nc.declare_dram_parameter("ohT", [128, T * 128], BF16, isOutput=False)
    posA = nc.declare_dram_parameter("posA", [128, Ppad // 16], I16, isOutput=False)
    posB = nc.declare_dram_parameter("posB", [128, Ppad // 16], I16, isOutput=False)
    we1 = nc.declare_dram_parameter("we1", [3 * H, H], F32, isOutput=False)
    we2 = nc.declare_dram_parameter("we2", [H, H], F32, isOutput=False)
    wn1 = nc.declare_dram_parameter("wn1", [2 * H, H], F32, isOutput=False)
    wn2 = nc.declare_dram_parameter("wn2", [H, H], F32, isOutput=False)
    be1c = nc.declare_dram_parameter("be1c", [128, 2], F32, isOutput=False)
    bn1c = nc.declare_dram_parameter("bn1c", [128, 2], F32, isOutput=False)
    bn2b = nc.declare_dram_parameter("bn2b", [128, H], F32, isOutput=False)

    e2_out = nc.declare_dram_parameter("e2_out", [2 * Ppad, H], F32, isOutput=True)
    oh_out = nc.declare_dram_parameter("oh_out", [BAND, H], F32, isOutput=True)
    HW, SCRA, SCRB = cfg["HW"], cfg["SCRA"], cfg["SCRB"]
    scrA = nc.declare_dram_parameter("scrA", [SCRA, H], BF16, isOutput=True)
    scrB = nc.declare_dram_parameter("scrB", [SCRB, H], BF16, isOutput=True)

    partial = nc.dram_tensor("partial", [NPAD, H], BF16)
    rs_out = nc.dram_tensor("rs_out", [BAND, H], BF16)

    GB = 2 if "gb2" in DBG else 4   # subtiles per gather batch

    with ExitStack() as ctx:
        tc = ctx.enter_context(tile.TileContext(nc))

        const = ctx.enter_context(tc.tile_pool(name="const", bufs=1))
        wpool = ctx.enter_context(tc.tile_pool(name="wpool", bufs=1))

        # ---- constants / aux ----
        identb = const.tile([128, 128], BF16)
        make_identity(nc, identb[:])

        posA_sb = const.tile([128, Ppad // 16], I16)
        nc.sync.dma_start(posA_sb[:], posA[:])
        posB_sb = const.tile([128, Ppad // 16], I16)
        nc.sync.dma_start(posB_sb[:], posB[:])
        gia_sb = const.tile([128, Ppad // 16], I16)
        nc.sync.dma_start(gia_sb[:], gidx_a[:])
        hbw = const.tile([128, WINB, H], BF16)
        nc.sync.dma_start(
            hbw[:], hbb[:].rearrange("(s p) h -> p s h", s=WINB, p=128))
        be1_sb = const.tile([128, 2], F32)
        nc.sync.dma_start(be1_sb[:], be1c[:])
        bn1_sb = const.tile([128, 2], F32)
        nc.sync.dma_start(bn1_sb[:], bn1c[:])
        bn2_sb = const.tile([128, H], F32)
        nc.sync.dma_start(bn2_sb[:], bn2b[:])

        band_acc = const.tile([128, WINB, H], F32)

        # ---- weight prep (bf16) ----
        wtmp = wpool.tile([128, 6, H], F32)
        for k in range(6):
            nc.sync.dma_start(wtmp[:, k, :], we1[128 * k:128 * (k + 1), :])
        w1s = wpool.tile([128, 4, H], BF16)   # kc 0,1 = (W1a+W1b)/2 ; kc 2,3 = W1c/2
        wdf = wpool.tile([128, 2, H], BF16)
        wsc = wpool.tile([128, 2, H], F32)
        for kc in range(2):
            nc.vector.tensor_add(wsc[:, kc, :], wtmp[:, kc, :], wtmp[:, 2 + kc, :])
            nc.scalar.activation(w1s[:, kc, :], wsc[:, kc, :],
                                 mybir.ActivationFunctionType.Copy, scale=0.5)
            nc.scalar.activation(w1s[:, 2 + kc, :], wtmp[:, 4 + kc, :],
                                 mybir.ActivationFunctionType.Copy, scale=0.5)
        wsc2 = wpool.tile([128, 2, H], F32)
        for kc in range(2):
            nc.vector.tensor_sub(wsc2[:, kc, :], wtmp[:, kc, :], wtmp[:, 2 + kc, :])
            nc.scalar.activation(wdf[:, kc, :], wsc2[:, kc, :],
                                 mybir.ActivationFunctionType.Copy, scale=0.5)
        w2s = wpool.tile([128, 2, H], BF16)
        wn1s = wpool.tile([128, 4, H], BF16)
        wn2s = wpool.tile([128, 2, H], BF16)
        wtmp2 = wpool.tile([128, 4, H], F32)
        for kc in range(2):
            nc.sync.dma_start(wtmp2[:, kc, :], we2[128 * kc:128 * (kc + 1), :])
            nc.scalar.activation(w2s[:, kc, :], wtmp2[:, kc, :],
                                 mybir.ActivationFunctionType.Copy)
        wtmp3 = wpool.tile([128, 4, H], F32)
        for kc in range(4):
            nc.sync.dma_start(wtmp3[:, kc, :], wn1[128 * kc:128 * (kc + 1), :])
            nc.scalar.activation(wn1s[:, kc, :], wtmp3[:, kc, :],
                                 mybir.ActivationFunctionType.Copy)
        wtmp4 = wpool.tile([128, 2, H], F32)
        for kc in range(2):
            nc.sync.dma_start(wtmp4[:, kc, :], wn2[128 * kc:128 * (kc + 1), :])
            nc.scalar.activation(wn2s[:, kc, :], wtmp4[:, kc, :],
                                 mybir.ActivationFunctionType.Copy)

        # ================= phase A: pair pipeline =================
        with (
            tc.tile_pool(name="ght", bufs=2) as ghtp,
            tc.tile_pool(name="ework", bufs=3) as ep,
            tc.tile_pool(name="work", bufs=3) as wk,
            tc.tile_pool(name="msg", bufs=3) as msgp,
            tc.tile_pool(name="ohp", bufs=2) as ohp,
            tc.tile_pool(name="ps_t", bufs=2, space="PSUM") as ps_t,
            tc.tile_pool(name="ps_d", bufs=1, space="PSUM") as ps_d,
            tc.tile_pool(name="ps_mf", bufs=2, space="PSUM") as ps_mf,
            tc.tile_pool(name="ps_tr", bufs=1, space="PSUM") as ps_tr,
            tc.tile_pool(name="ps_w", bufs=1, space="PSUM") as ps_w,
            tc.tile_pool(name="ps_hb", bufs=1, space="PSUM") as ps_hb,
        ):
            ght_a = None
            ohT_sb = None
            mf_g = None
            psw = None
            def wl_of(t):
                return int(np.searchsorted(cumE, t, side="right") - 1)
            n_mt = (T + 1) // 2
            for mt in range(n_mt):
                t0 = 2 * mt
                nsub = min(2, T - t0)         # subtiles in this macrotile
                PW = 128 * nsub               # pairs in macrotile

                if t0 % GB == 0:
                    nb = min(GB, T - t0)      # subtiles in gather batch
                    ght_a = ghtp.tile([128, 2, 128 * nb], BF16, tag="ga")
                    nc.gpsimd.dma_gather(
                        out_ap=ght_a[:],
                        in_ap=h_bf[:],
                        idxs_ap=gia_sb[:, 8 * t0: 8 * (t0 + nb)],
                        num_idxs=128 * nb,
                        num_idxs_reg=128 * nb,
                        elem_size=H,
                        transpose=True,
                    )
                    ohT_sb = ohp.tile([128, 128 * nb], BF16, tag="oht")
                    nc.sync.dma_start(ohT_sb[:], ohT[:, 128 * t0:128 * (t0 + nb)])
                    mf_g = msgp.tile([128, GB, H], BF16, tag="mfb")
                    ohE_sb = ohp.tile([128, 128 * nb], BF16, tag="ohe")
                    nc.sync.dma_start(ohE_sb[:], ohE[:, 128 * t0:128 * (t0 + nb)])
                goff = 128 * (t0 % GB)

                # e rows for macrotile: [128, nsub, 2, H]
                e_sb = ep.tile([128, 2, 2, H], F32, tag="e")
                src = e_in[512 * mt: 512 * mt + 256 * nsub, :]
                nc.sync.dma_start(
                    e_sb[:, 0:nsub, :, :],
                    src.rearrange("(s p c) h -> p s c h", s=nsub, p=128, c=2),
                )

                # sE = e0+e1 (bf16; the 0.5 is folded into W1c)
                se_b = wk.tile([128, 2, H], BF16, tag="se")
                nc.vector.tensor_add(se_b[:, 0:nsub, :], e_sb[:, 0:nsub, 0, :],
                                     e_sb[:, 0:nsub, 1, :])

                # seT via PE transpose
                seT = wk.tile([128, 2, PW], BF16, tag="seT")
                for s in range(nsub):
                    for cch in range(2):
                        pst = ps_tr.tile([128, 128], BF16, space="PSUM", tag="tr")
                        nc.tensor.transpose(pst[:], se_b[:, s, 128 * cch:128 * (cch + 1)], identb[:])
                        nc.scalar.activation(seT[:, cch, 128 * s:128 * (s + 1)], pst[:],
                                             mybir.ActivationFunctionType.Copy)

                # hb via one-hot expansion from the band window (dst_even is local)
                phb = ps_hb.tile([128, 2, PW], F32, space="PSUM", tag="hb")
                for s in range(nsub):
                    t = t0 + s
                    wlt = wl_of(t)
                    for cch in range(2):
                        nc.tensor.matmul(
                            phb[:, cch, 128 * s:128 * (s + 1)],
                            lhsT=hbw[:, wlt, 128 * cch:128 * (cch + 1)],
                            rhs=ohT_sb[:, 128 * (t % GB):128 * (t % GB + 1)],
                            start=True, stop=True)
                # u/v
                ga = ght_a[:, :, goff:goff + PW]
                uT = wk.tile([128, 2, PW], BF16, tag="uT")
                vT = wk.tile([128, 2, PW], BF16, tag="vT")
                nc.vector.tensor_add(uT[:], ga, phb[:, :, 0:PW])
                nc.vector.tensor_sub(vT[:], ga, phb[:, :, 0:PW])

                # GEMM1a: t = u @ Wsum' + sE @ (W1c/2)   -> [Hout(2x128), PW]
                pt = ps_t.tile([128, 2, PW], F32, space="PSUM", tag="pt")
                for mc in range(2):
                    for kc in range(4):
                        rhs = uT[:, kc, :] if kc < 2 else seT[:, kc - 2, :]
                        nc.tensor.matmul(pt[:, mc, :], lhsT=w1s[:, kc, 128 * mc:128 * (mc + 1)],
                                         rhs=rhs, start=(kc == 0), stop=(kc == 3))
                # GEMM1b: d = v @ Wdiff'
                pd = ps_d.tile([128, 2, PW], F32, space="PSUM", tag="pd")
                for mc in range(2):
                    for kc in range(2):
                        nc.tensor.matmul(pd[:, mc, :], lhsT=wdf[:, kc, 128 * mc:128 * (mc + 1)],
                                         rhs=vT[:, kc, :], start=(kc == 0), stop=(kc == 1))

                pdc = wk.tile([128, 2, PW], F32, tag="pdc")
                nc.scalar.activation(pdc[:], pd[:, :, 0:PW],
                                     mybir.ActivationFunctionType.Copy)
                zab = wk.tile([128, 2, PW], F32, tag="zab")
                zba = wk.tile([128, 2, PW], F32, tag="zba")
                nc.vector.tensor_add(zab[:], pt[:, :, 0:PW], pdc[:])
                nc.vector.tensor_sub(zba[:], pt[:, :, 0:PW], pdc[:])
                gab = wk.tile([128, 2, PW], F32, tag="gab")
                gba = wk.tile([128, 2, PW], F32, tag="gba")
                for mc in range(2):
                    _silu_emit(nc, wk, gab[:, mc, :], zab[:, mc, :],
                               be1_sb[:, mc:mc + 1], "sa")
                    _silu_emit(nc, wk, gba[:, mc, :], zba[:, mc, :],
                               be1_sb[:, mc:mc + 1], "sb")
                gdf = wk.tile([128, 2, PW], BF16, tag="gdf")
                nc.vector.tensor_sub(gdf[:], gab[:], gba[:])

                e2_sb = ep.tile([128, 2, 2, H], F32, tag="e2")
                pmf = ps_mf.tile([128, 2, H], F32, space="PSUM", tag="mf")
                mf_b = mf_g[:, t0 % GB:t0 % GB + nsub, :]
                for s in range(nsub):
                    # GEMM2: mf = g @ W_e2  -> [128 pairs, H]
                    for kc in range(2):
                        nc.tensor.matmul(pmf[:, s, :], lhsT=gdf[:, kc, 128 * s:128 * (s + 1)],
                                         rhs=w2s[:, kc, :], start=(kc == 0), stop=(kc == 1))
                nc.vector.tensor_copy(mf_b, pmf[:, 0:nsub, :])
                nc.vector.tensor_add(e2_sb[:, 0:nsub, 0, :], e_sb[:, 0:nsub, 0, :],
                                     pmf[:, 0:nsub, :])
                nc.vector.tensor_sub(e2_sb[:, 0:nsub, 1, :], e_sb[:, 0:nsub, 1, :],
                                     pmf[:, 0:nsub, :])
                for s in range(nsub):
                    t = t0 + s
                    # even scatter (fused, +1 one-hot from host)
                    wl = int(np.searchsorted(cumE, t, side="right") - 1)
                    if "noeven" in DBG:
                        wl = None
                    first = (wl is not None) and (t == cumE[wl])
                    last = (wl is not None) and (t == cumE[wl + 1] - 1)
                    if first:
                        psw = ps_w.tile([128, H], F32, space="PSUM", tag="win")
                    if wl is not None:
                        nc.tensor.matmul(psw[:], lhsT=ohE_sb[:, 128 * (t % GB):128 * (t % GB + 1)],
                                         rhs=mf_g[:, t % GB, :],
                                         start=first, stop=last, skip_group_check=True)
                        if last:
                            nc.vector.tensor_copy(band_acc[:, wl, :], psw[:])

                # odd scatter: per gather-group, two halves via dma_scatter_add
                if "noscat" not in DBG and ((t0 + nsub) % GB == 0 or (t0 + nsub) == T):
                    g0 = (t0 // GB) * GB
                    ng = t0 + nsub - g0
                    for scrh, posh in ((scrA, posA_sb), (scrB, posB_sb)):
                        nc.gpsimd.dma_scatter_add(
                            scrh[:],
                            mf_g[:, 0:ng, :],
                            posh[:, 8 * g0: 8 * (g0 + ng)],
                            128 * ng,
                            128 * ng,
                            H,
                        )

                dstv = e2_out[512 * mt: 512 * mt + 256 * nsub, :]
                nc.sync.dma_start(
                    dstv.rearrange("(s p c) h -> p s c h", s=nsub, p=128, c=2),
                    e2_sb[:, 0:nsub, :, :],
                )

        # ================= phase B: odd window reduction =================
        if "nophaseb" not in DBG:
         with (
            tc.tile_pool(name="scrd", bufs=3) as scrp,
            tc.tile_pool(name="pout", bufs=2) as poutp,
            tc.tile_pool(name="ps_b", bufs=2, space="PSUM") as ps_b,
         ):
            SB_ = 8   # scratch tiles per load batch
            WB = 8    # windows per output DMA batch
            w_of_st = np.zeros(TO, np.int64)
            for w in range(NWIN):
                w_of_st[cumO[w]:cumO[w + 1]] = w
            TA = int(cumO[HW])
            sc_sb = None
            oh_sb = None
            pb = None
            psb = None
            batch_lo = 0
            for st in range(TO):
                if st == batch_lo:
                    # batch must not straddle the scratch halves
                    lim = TA if st < TA else TO
                    nb3 = min(SB_, lim - st)
                    batch_lo = st + nb3
                    if st < TA:
                        srct, off = scrA, st
                    else:
                        srct, off = scrB, st - TA
                    sc_sb = scrp.tile([128, SB_, H], BF16, tag="sc")
                    nc.sync.dma_start(
                        sc_sb[:, 0:nb3, :],
                        srct[128 * off:128 * (off + nb3), :].rearrange(
                            "(s p) h -> p s h", s=nb3, p=128),
                    )
                    b0 = st
                    oh_sb = scrp.tile([128, SB_ * 128], BF16, tag="ohb")
                    nc.sync.dma_start(oh_sb[:, 0:128 * nb3],
                                      ohO[:, 128 * st:128 * (st + nb3)])
                w = int(w_of_st[st])
                if st == cumO[w]:
                    psb = ps_b.tile([128, H], F32, space="PSUM", tag="bw")
                nc.tensor.matmul(psb[:], lhsT=oh_sb[:, 128 * (st - b0):128 * (st - b0 + 1)],
                                 rhs=sc_sb[:, st - b0, :],
                                 start=(st == cumO[w]), stop=(st == cumO[w + 1] - 1),
                                 skip_group_check=True)
                if st == cumO[w + 1] - 1:
                    if w % WB == 0:
                        pb = poutp.tile([128, WB, H], BF16, tag="pb")
                    nc.scalar.activation(pb[:, w % WB, :], psb[:],
                                         mybir.ActivationFunctionType.Copy)
                    if w % WB == WB - 1:
                        wg = w // WB
                        dst = partial[128 * WB * wg: 128 * WB * (wg + 1), :]
                        nc.sync.dma_start(
                            dst.rearrange("(s p) h -> p s h", s=WB, p=128),
                            pb[:],
                        )

        # ================= reduce-scatter =================
        if "nors" not in DBG:
         nc.gpsimd.collective_compute(
            "ReduceScatter",
            mybir.AluOpType.add,
            replica_groups=[list(range(NC))],
            ins=[partial[:]],
            outs=[rs_out[:]],
         )

        # ================= phase C: node MLP on band =================
        if "nophasec" not in DBG:
         with (
            tc.tile_pool(name="nfw", bufs=3) as nf,
            tc.tile_pool(name="nfT", bufs=1) as nfT,
            tc.tile_pool(name="ps_z", bufs=2, space="PSUM") as ps_z,
            tc.tile_pool(name="ps_o", bufs=2, space="PSUM") as ps_o,
            tc.tile_pool(name="ps_tr", bufs=2, space="PSUM") as ps_tr,
        ):
            nix = nf.tile([128, BAND // 16], I16, tag="nix")
            nc.sync.dma_start(nix[:], nidx[:])
            GC = 256 if BAND >= 256 else 128
            hTs = []
            for i in range(BAND // GC):
                hTc = nfT.tile([128, 2, GC], BF16, tag=f"hT{i}")
                nc.gpsimd.dma_gather(
                    out_ap=hTc[:], in_ap=h_bf[:],
                    idxs_ap=nix[:, (GC // 16) * i:(GC // 16) * (i + 1)],
                    num_idxs=GC, num_idxs_reg=GC, elem_size=H, transpose=True,
                )
                hTs.append(hTc)
            aggT = nfT.tile([128, 2, BAND], BF16, tag="aggT")
            agg_b = nfT.tile([128, WINB, H], BF16, tag="aggb")
            rs_sb = nfT.tile([128, WINB, H], BF16, tag="rs")
            nc.sync.dma_start(
                rs_sb[:], rs_out[:].rearrange("(s p) h -> p s h", s=WINB, p=128))
            rs_f = nfT.tile([128, WINB, H], F32, tag="rsf")
            nc.scalar.activation(rs_f[:], rs_sb[:], mybir.ActivationFunctionType.Copy)
            nc.vector.tensor_add(agg_b[:], band_acc[:], rs_f[:])
            for i in range(WINB):
                for cch in range(2):
                    ptr = ps_tr.tile([128, 128], BF16, space="PSUM", tag="ntr")
                    nc.tensor.transpose(ptr[:], agg_b[:, i, 128 * cch:128 * (cch + 1)], identb[:])
                    nc.scalar.activation(aggT[:, cch, 128 * i:128 * (i + 1)], ptr[:],
                                         mybir.ActivationFunctionType.Copy)

            NF = min(256, BAND)
            for nfc in range(BAND // NF):
                pz = ps_z.tile([128, 2, NF], F32, space="PSUM", tag="pz")
                for mc in range(2):
                    for kc in range(4):
                        rhs = (hTs[(NF * nfc) // GC][:, kc - 2 if False else kc, (NF * nfc) % GC:(NF * nfc) % GC + NF] if kc < 2
                               else aggT[:, kc - 2, NF * nfc:NF * (nfc + 1)])
                        nc.tensor.matmul(pz[:, mc, :], lhsT=wn1s[:, kc, 128 * mc:128 * (mc + 1)],
                                         rhs=rhs, start=(kc == 0), stop=(kc == 3))
                gn = nf.tile([128, 2, NF], BF16, tag="gn")
                for mc in range(2):
                    _silu_emit(nc, nf, gn[:, mc, :], pz[:, mc, :],
                               bn1_sb[:, mc:mc + 1], "sn")
                for i4 in range(NF // 128):
                    row0 = NF * nfc + 128 * i4
                    po = ps_o.tile([128, H], F32, space="PSUM", tag="po")
                    for kc in range(2):
                        nc.tensor.matmul(po[:], lhsT=gn[:, kc, 128 * i4:128 * (i4 + 1)],
                                         rhs=wn2s[:, kc, :], start=(kc == 0), stop=(kc == 1))
                    hb_sb = nf.tile([128, H], F32, tag="hb")
                    nc.sync.dma_start(hb_sb[:], hband[row0:row0 + 128, :])
                    o1 = nf.tile([128, H], F32, tag="o1")
                    nc.vector.tensor_add(o1[:], po[:], hb_sb[:])
                    o2 = nf.tile([128, H], F32, tag="o2")
                    nc.vector.tensor_add(o2[:], o1[:], bn2_sb[:])
                    nc.sync.dma_start(oh_out[row0:row0 + 128, :], o2[:])
                    del hb_sb, o1, o2

    nc.compile()
    return nc


# ----------------------------------------------------------------------------
# entry point
# ----------------------------------------------------------------------------

def kernel(h, e, W_e1, b_e1, W_e2, b_e2, W_n1, b_n1, W_n2, b_n2, ei, _trace=False):
    h = np.asarray(h, dtype=np.float32)
    e = np.asarray(e, dtype=np.float32)
    ei_np = np.asarray(ei)
    W_e1 = np.asarray(W_e1, dtype=np.float32)
    b_e1 = np.asarray(b_e1, dtype=np.float32)
    W_e2 = np.asarray(W_e2, dtype=np.float32)
    W_n1 = np.asarray(W_n1, dtype=np.float32)
    b_n1 = np.asarray(b_n1, dtype=np.float32)
    W_n2 = np.asarray(W_n2, dtype=np.float32)
    b_n2 = np.asarray(b_n2, dtype=np.float32)
    b_e2 = np.asarray(b_e2, dtype=np.float32)

    h2, e2d = h[0], e[0]
    cfg, in_maps, asm = host_prep(h2, e2d, ei_np.astype(np.int64))

    be1c = np.ascontiguousarray(b_e1.reshape(2, 128).T)
    bn1c = np.ascontiguousarray(b_n1.reshape(2, 128).T)
    bn2b = np.tile(b_n2.reshape(1, H), (128, 1)).astype(np.float32)

    for m in in_maps:
        m.update(dict(we1=W_e1, we2=W_e2, wn1=W_n1, wn2=W_n2,
                      be1c=be1c, bn1c=bn1c, bn2b=bn2b))

    key = (cfg["T"], cfg["TO"], cfg["SCR"], cfg["capE"], cfg["capO"], os.environ.get("KDBG",""))
    if key not in _cache:
        _cache[key] = build(cfg)
    nc = _cache[key]

    res = run_bass_kernel_spmd(nc, in_maps, core_ids=list(range(NC)), trace=_trace)
    kernel._last_results = res

    out_h = np.empty((1, N, H), np.float32)
    e2 = np.empty((1, E, H), np.float32)
    for c in range(NC):
        r = res.results[c]
        lo, hi = BAND * c, min(BAND * (c + 1), N)
        out_h[0, lo:hi] = r["oh_out"][: hi - lo]
        pidx_pad, real = asm[c]
        pid = pidx_pad[real]
        ero = r["e2_out"].reshape(-1, 2, H)[real]
        e2[0, 2 * pid] = ero[:, 0]
        e2[0, 2 * pid + 1] = ero[:, 1]
    # add b_e2 to both sides?  No: b_e2 cancels in mf; e2 = e + msgs exactly.
    return out_h, e2
